# revision 39
# baseline (speedup 1.0000x reference)
"""Trainium2 Bass kernel for nn_MoECustomBasicBlock (moe_routing).

Strategy: data-parallel over batch, 1 sample per core (B=8 == n_cores).
Per core, the full block runs on device:
  g1/g2:  fp32 gate matmul -> top-32 (max8/match_replace) -> masked softmax
  conv1:  3x3 stride-2, x stored even/odd-row split across partitions so
          dy-taps pair into K=128 bf16 matmuls; fp32 PSUM accumulate
  ds:     1x1 stride-2 conv reads the even-row half of the same x tile
  conv2:  3x3 stride-1 over an SBUF-resident zero-padded bf16 out2 buffer
  gating/leaky: ACT-engine eviction with per-partition scale (gate value),
          leaky(v) = max(v, 0.2 v) in one scalar_tensor_tensor op
"""
import os
import sys

for _p in ("/opt/trn_rl_repo",):
    if _p not in sys.path and os.path.isdir(_p):
        sys.path.insert(0, _p)

import numpy as np

import concourse.bacc as bacc
import concourse.tile as tile
from concourse import mybir
from concourse.bass_utils import run_bass_kernel_spmd

F32 = mybir.dt.float32
BF16 = mybir.dt.bfloat16
NP_BF16 = mybir.dt.np(BF16)
ALU = mybir.AluOpType
ACT = mybir.ActivationFunctionType

B, CIN, COUT, H, W, E = 8, 64, 128, 192, 384, 128
HO, WO = H // 2, W // 2          # 96, 192
R = 16                           # output rows per chunk
NCH = HO // R                    # 6 chunks
WP = W + 2                       # padded input width (zero cols 0 and WP-1)
NGRP = HO // 2                   # 48 two-row groups
EPS = 1e-5
NEG = -1e30


def _emit_gate(nc, pools, embc, gw, gb, go_d, tag):
    """Gate: logits = emb @ W + b; top-32 mask; masked softmax.
    Returns the [COUT,1] fp32 per-partition gate column used at eviction."""
    gpool, pgate = pools
    pg = pgate.tile([1, COUT], F32, name=f"pg{tag}", tag="pg")
    nc.tensor.matmul(pg[:], embc[:], gw[:], start=True, stop=True)

    logits = gpool.tile([1, COUT], F32, name=f"logit{tag}", tag=f"logit{tag}")
    nc.vector.tensor_add(logits[:], pg[:], gb[:])

    scratch = gpool.tile([1, COUT], F32, name=f"scr{tag}", tag=f"scr{tag}")
    nc.vector.tensor_copy(scratch[:], logits[:])
    maxb = gpool.tile([1, 32], F32, name=f"maxb{tag}", tag=f"maxb{tag}")
    for q in range(4):
        nc.vector.max(out=maxb[:, 8 * q:8 * q + 8], in_=scratch[:])
        if q < 3:
            nc.vector.match_replace(
                out=scratch[:], in_to_replace=maxb[:, 8 * q:8 * q + 8],
                in_values=scratch[:], imm_value=NEG)

    t32 = maxb[:, 31:32]
    m0 = maxb[:, 0:1]
    nm = gpool.tile([1, 1], F32, name=f"nm{tag}", tag=f"nm{tag}")
    nc.vector.tensor_scalar_mul(nm[:], m0, -1.0)

    e = gpool.tile([1, COUT], F32, name=f"e{tag}", tag=f"e{tag}")
    nc.scalar.activation(e[:], logits[:], ACT.Exp, bias=nm[:])
    mask = gpool.tile([1, COUT], F32, name=f"mask{tag}", tag=f"mask{tag}")
    nc.vector.tensor_scalar(mask[:], logits[:], t32, None, ALU.is_ge)
    em = gpool.tile([1, COUT], F32, name=f"em{tag}", tag=f"em{tag}")
    nc.vector.tensor_mul(em[:], e[:], mask[:])

    s = gpool.tile([1, 1], F32, name=f"s{tag}", tag=f"s{tag}")
    nc.vector.reduce_sum(s[:], em[:], axis=mybir.AxisListType.X)
    r = gpool.tile([1, 1], F32, name=f"r{tag}", tag=f"r{tag}")
    nc.vector.reciprocal(r[:], s[:])

    grow = gpool.tile([1, COUT], F32, name=f"grow{tag}", tag=f"grow{tag}")
    nc.vector.tensor_scalar_mul(grow[:], em[:], r[:])

    gcol = gpool.tile([COUT, 1], F32, name=f"gcol{tag}", tag=f"gcol{tag}")

    def finish():
        nc.scalar.dma_start(go_d[:], grow[:])
        # transpose [1,COUT] -> [COUT,1] on PE (matmul with scalar 1) — an
        # SBUF->SBUF transpose DMA would force an xbar-mode drain; deferred
        # into the conv1 stream so PE doesn't stall in-order on the chain
        one = gpool.tile([1, 1], F32, name=f"one{tag}", tag=f"one{tag}")
        nc.vector.memset(one[:], 1.0)
        pgc = pgate.tile([COUT, 1], F32, name=f"pgc{tag}", tag="pg")
        nc.tensor.matmul(pgc[:], grow[:], one[:], start=True, stop=True)
        nc.vector.tensor_copy(gcol[:], pgc[:])

    return gcol, finish


def _emit(tc, nc, ins, outs, ctx):
    x_d = ins["x"]
    xr = x_d.rearrange("c (h t) w -> c h t w", t=2)  # even rows t=0, odd t=1

    wpool = ctx.enter_context(tc.tile_pool(name="w", bufs=1))
    gpool = ctx.enter_context(tc.tile_pool(name="g", bufs=2))
    xpool = ctx.enter_context(tc.tile_pool(name="x", bufs=1))
    opool = ctx.enter_context(tc.tile_pool(name="o", bufs=1))
    tpool = ctx.enter_context(tc.tile_pool(name="t", bufs=4))
    spool = ctx.enter_context(tc.tile_pool(name="s", bufs=4))
    pc1 = ctx.enter_context(tc.tile_pool(name="pc1", bufs=3, space="PSUM"))
    pds = ctx.enter_context(tc.tile_pool(name="pds", bufs=2, space="PSUM"))
    pc2 = ctx.enter_context(tc.tile_pool(name="pc2", bufs=2, space="PSUM"))
    pgate = ctx.enter_context(tc.tile_pool(name="pg", bufs=1, space="PSUM"))

    # ---- x buffers + first chunk load go first: x0 is on the critical path
    land = [xpool.tile([128, R + 1, WP], F32, name=f"land{i}", tag=f"land{i}") for i in range(2)]
    # zero only what the x DMAs never write (pad cols; chunk-0 odd slot 0):
    # disjoint regions, so the loads don't serialize behind these memsets
    for i in range(2):
        nc.gpsimd.memset(land[i][:, :, 0:1], 0.0)
        nc.gpsimd.memset(land[i][:, :, W + 1:W + 2], 0.0)
        # even half never fills slot R; zero it so the full-tile cast is
        # defined (the bf16 values there are never consumed)
        nc.gpsimd.memset(land[i][0:64, R:R + 1, 1:W + 1], 0.0)
    nc.gpsimd.memset(land[0][64:128, 0:1, 1:W + 1], 0.0)

    def dma_x(k):
        r0 = R * k
        ld = land[k % 2]
        # even input rows 2(r0+j), j=0..R-1 -> partitions 0:64, slots 0..R-1
        nc.sync.dma_start(ld[0:64, 0:R, 1:W + 1],
                          xr[:, r0:r0 + R, 0:1, :].squeeze())
        # odd input rows 2(r0+j)-1 -> partitions 64:128, slot j
        if k == 0:
            nc.sync.dma_start(ld[64:128, 1:R + 1, 1:W + 1],
                              xr[:, 0:R, 1:2, :].squeeze())
        else:
            nc.sync.dma_start(ld[64:128, 0:R + 1, 1:W + 1],
                              xr[:, r0 - 1:r0 + R, 1:2, :].squeeze())

    dma_x(0)

    # ---- weights (transposed [co, dx, k] for on-device channel gather) ----
    c1aT = wpool.tile([128, 3, 128], BF16, name="c1aT", tag="c1aT")
    c1bT = wpool.tile([128, 3, 128], BF16, name="c1bT", tag="c1bT")
    dsw = wpool.tile([64, COUT], BF16, name="dsw", tag="dsw")
    c2w = wpool.tile([128, 9, COUT], BF16, name="c2w", tag="c2w")
    dsb = wpool.tile([COUT, 1], F32, name="dsb", tag="dsb")
    iot = wpool.tile([128, 96], F32, name="iot", tag="iot")
    tri = wpool.tile([128, 128], BF16, name="tri", tag="tri")
    for t, d in ((c1aT, ins["c1aT"]), (c1bT, ins["c1bT"]), (dsw, ins["dsw"]),
                 (c2w, ins["c2w"]), (dsb, ins["dsb"]), (iot, ins["iota96"]),
                 (tri, ins["tri"])):
        nc.sync.dma_start(t[:], d[:])

    embc = wpool.tile([E, 1], F32, name="embc", tag="embc")
    nc.scalar.dma_start(embc[:], ins["emb"][:])

    xbf = [xpool.tile([128, R + 1, WP], BF16, name=f"xbf{i}", tag=f"xbf{i}")
           for i in range(2)]
    def cast_x(k):
        nc.vector.tensor_copy(xbf[k % 2][:], land[k % 2][:])

    cast_x(0)

    gwt, gbt = {}, {}
    for t in ("1", "2"):
        gwt[t] = gpool.tile([E, COUT], F32, name=f"gw{t}", tag=f"gw{t}")
        gbt[t] = gpool.tile([1, COUT], F32, name=f"gb{t}", tag=f"gb{t}")
        nc.scalar.dma_start(gwt[t][:], ins[f"g{t}w"][:])
        nc.scalar.dma_start(gbt[t][:], ins[f"g{t}b"][:])
    g1col, g1fin = _emit_gate(nc, (gpool, pgate), embc, gwt["1"], gbt["1"],
                              outs["g1o"], "1")
    g2col, g2fin = _emit_gate(nc, (gpool, pgate), embc, gwt["2"], gbt["2"],
                              outs["g2o"], "2")

    # ---- persistent buffers ----
    # out2c: compacted out2 — 3 partition blocks of the 32 g1-selected
    # channels, block q pre-shifted so tap dx=q reads cols 1..192; block q
    # stores out col v at storage col v+2-q
    out2c = opool.tile([96, HO + 2, WO + 2], BF16, name="out2c", tag="out2c")
    ident = [opool.tile([COUT, R, WO], F32, name=f"ident{i}", tag=f"ident{i}") for i in range(2)]

    # zero padding: boundary rows + per-block gutter cols inside read range
    nc.gpsimd.memset(out2c[:, 0:1, :], 0.0)
    nc.gpsimd.memset(out2c[:, HO + 1:HO + 2, :], 0.0)
    nc.gpsimd.memset(out2c[0:32, :, 1:2], 0.0)
    nc.gpsimd.memset(out2c[64:96, :, WO:WO + 1], 0.0)

    def conv2_group(j):
        """One 2-row output group of conv2 + residual + final leaky."""
        h = 2 * j
        p2 = pc2.tile([COUT, 2, WO], F32, name="p2", tag="p2")
        for p in range(3):
            rhs = out2c[0:96, h + p:h + p + 2, 1:WO + 1]
            nc.tensor.matmul(p2[:], c2wp[:, p, :], rhs,
                             start=(p == 0), stop=(p == 2))
        a = tpool.tile([COUT, 2, WO], F32, name="a", tag="a")
        # a = psum * g2   (per-partition gate scale on ACT engine)
        nc.scalar.activation(a[:], p2[:], ACT.Copy, scale=g2col[:])
        # a = leaky(a)
        nc.vector.scalar_tensor_tensor(a[:], a[:], 0.2, a[:], ALU.mult, ALU.max)
        # a = a + ident
        c = j // 8
        idt = ident[c % 2][:, h - 16 * c:h - 16 * c + 2, :]
        nc.gpsimd.tensor_add(a[:], a[:], idt)
        st = spool.tile([COUT, 2, WO], F32, name="st", tag="st")
        nc.vector.scalar_tensor_tensor(st[:], a[:], 0.2, a[:], ALU.mult, ALU.max)
        nc.sync.dma_start(outs["out"][:, h:h + 2, :], st[:])

    c1ac = wpool.tile([128, 3, 32], BF16, name="c1ac", tag="c1ac")
    c1bc = wpool.tile([128, 3, 32], BF16, name="c1bc", tag="c1bc")
    c2wp = wpool.tile([96, 3, COUT], BF16, name="c2wp", tag="c2wp")
    g1c = gpool.tile([32, 1], F32, name="g1c", tag="g1c")

    def emit_gather():
        # E1_3 = [E1|E1|E1]: one-hot over the 32 g1-selected channels,
        # built from rank = tri-matmul prefix-count of the gate mask
        maskf = gpool.tile([128, 1], F32, name="maskf", tag="maskf")
        nc.vector.tensor_scalar(maskf[:], g1col[:], 0.0, None, ALU.is_gt)
        maskb = gpool.tile([128, 1], BF16, name="maskb", tag="maskb")
        nc.vector.tensor_scalar(maskb[:], g1col[:], 0.0, None, ALU.is_gt)
        csp = pgate.tile([128, 1], F32, name="csp", tag="pg")
        nc.tensor.matmul(csp[:], tri[:], maskb[:], start=True, stop=True)
        csm1 = gpool.tile([128, 1], F32, name="csm1", tag="csm1")
        nc.vector.tensor_scalar(csm1[:], csp[:], -1.0, None, ALU.add)
        e1pre = gpool.tile([128, 96], F32, name="e1pre", tag="e1pre")
        nc.vector.tensor_scalar(e1pre[:], iot[:], csm1[:], None, ALU.is_equal)
        e13 = gpool.tile([128, 96], BF16, name="e13", tag="e13")
        nc.vector.tensor_scalar(e13[:], e1pre[:], maskf[:], None, ALU.mult)
        e1f = gpool.tile([128, 32], F32, name="e1f", tag="e1f")
        nc.vector.tensor_scalar(e1f[:], e1pre[:, 0:32], maskf[:], None,
                                ALU.mult)
        # compacted gate values (exact f32 gather via one-hot matmul)
        gcp = pgate.tile([32, 1], F32, name="gcp", tag="pg")
        nc.tensor.matmul(gcp[:], e1f[:], g1col[:], start=True, stop=True)
        nc.vector.tensor_copy(g1c[:], gcp[:])
        # conv1 lhsT columns gathered to the 32 selected out-channels
        for dx in range(3):
            pa = pgate.tile([128, 32], F32, name="pa", tag="pg")
            nc.tensor.matmul(pa[:], c1aT[:, dx, :], e13[:, 0:32],
                             start=True, stop=True)
            nc.scalar.activation(c1ac[:, dx, :], pa[:], ACT.Copy)
            pb = pgate.tile([128, 32], F32, name="pb", tag="pg")
            nc.tensor.matmul(pb[:], c1bT[:, dx, :], e13[:, 0:32],
                             start=True, stop=True)
            nc.scalar.activation(c1bc[:, dx, :], pb[:], ACT.Copy)
        # conv2 lhsT: rows gathered to selected in-channels, packed so one
        # K=96 matmul per dy covers all three dx taps. Per-block one-hots
        # with disjoint output-partition ranges accumulate into one psum so
        # every PSUM eviction is a full-tile base-0 read (sliced/offset PSUM
        # reads by ACT diverged on real HW).
        e1q3 = [gpool.tile([128, 96], BF16, name=f"e1q{q}", tag=f"e1q{q}")
                for q in range(3)]
        for q in range(3):
            nc.vector.memset(e1q3[q][:], 0.0)
            nc.vector.tensor_copy(e1q3[q][:, 32 * q:32 * q + 32],
                                  e13[:, 32 * q:32 * q + 32])
        for p in range(3):
            pp = pc2.tile([96, COUT], F32, name="pp", tag="p2")
            for q in range(3):
                nc.tensor.matmul(pp[:], e1q3[q][:], c2w[:, 3 * p + q, :],
                                 start=(q == 0), stop=(q == 2))
            nc.scalar.activation(c2wp[:, p, :], pp[:], ACT.Copy)

    next_j = 0
    for k in range(NCH):
        r0 = R * k
        xb = xbf[k % 2]
        if k + 1 < NCH:
            dma_x(k + 1)

        for g in range(R // 2):
            if k == 0 and g == 0:
                g1fin()
                emit_gather()
            if k == 0 and g == 3:
                g2fin()
            i = 2 * g
            h = r0 + i  # absolute output row
            # conv1: taps (dy1,dy0) K=128 paired + dy2 K=64, x3 dx each
            # M=32: only the g1-selected output channels are computed
            p1 = pc1.tile([32, 2, WO], F32, name="p1", tag="p1")
            for dx in range(3):
                rhs = xb[0:128, i:i + 2, dx:dx + W].rearrange(
                    "p r (w t) -> p r w t", t=2)[:, :, :, 0:1]
                nc.tensor.matmul(p1[:], c1ac[:, dx, :], rhs,
                                 start=(dx == 0), stop=False)
            for dx in range(3):
                rhs = xb[64:128, i + 1:i + 3, dx:dx + W].rearrange(
                    "p r (w t) -> p r w t", t=2)[:, :, :, 0:1]
                nc.tensor.matmul(p1[:], c1bc[64:128, dx, :], rhs,
                                 start=False, stop=(dx == 2))
            t1 = tpool.tile([32, 2, WO], F32, name="t1", tag="t1")
            nc.scalar.activation(t1[:], p1[:], ACT.Copy, scale=g1c[:])
            nc.vector.scalar_tensor_tensor(
                out2c[0:32, h + 1:h + 3, 2:WO + 2], t1[:], 0.2, t1[:],
                ALU.mult, ALU.max)


            # downsample 1x1 stride-2: even rows, even cols
            pd = pds.tile([COUT, 2, WO], F32, name="pd", tag="pd")
            rhs = xb[0:64, i:i + 2, 1:W + 1].rearrange(
                "p r (w t) -> p r w t", t=2)[:, :, :, 0:1]
            nc.tensor.matmul(pd[:], dsw[:], rhs, start=True, stop=True)
            nc.scalar.activation(ident[k % 2][:, i:i + 2, :], pd[:],
                                 ACT.Identity, bias=dsb[:])

            if g in (3, 7):
                # replicate block 0 into the dx-shifted blocks, batched per
                # half-chunk to keep HWDGE issue count low
                hh = 1 if g == 7 else 0
                ra, rb = r0 + 8 * hh + 1, r0 + 8 * hh + 9
                nc.sync.dma_start(out2c[32:64, ra:rb, 1:WO + 1],
                                  out2c[0:32, ra:rb, 2:WO + 2])
                nc.scalar.dma_start(out2c[64:96, ra:rb, 0:WO],
                                    out2c[0:32, ra:rb, 2:WO + 2])
                # conv2 groups fully covered by the replicated rows
                while next_j <= 8 * k + 4 * hh + 2:
                    conv2_group(next_j)
                    next_j += 1
        if k + 1 < NCH:
            cast_x(k + 1)
    while next_j < NGRP:
        conv2_group(next_j)
        next_j += 1


def build_nc():
    nc = bacc.Bacc("TRN2", target_bir_lowering=False, debug=False)
    d = nc.dram_tensor
    ins = {
        "x": d("x", (CIN, H, W), F32, kind="ExternalInput").ap(),
        "emb": d("emb", (E, 1), F32, kind="ExternalInput").ap(),
        "c1aT": d("c1aT", (128, 3, 128), BF16, kind="ExternalInput").ap(),
        "c1bT": d("c1bT", (128, 3, 128), BF16, kind="ExternalInput").ap(),
        "iota96": d("iota96", (128, 96), F32, kind="ExternalInput").ap(),
        "tri": d("tri", (128, 128), BF16, kind="ExternalInput").ap(),
        "dsw": d("dsw", (64, COUT), BF16, kind="ExternalInput").ap(),
        "c2w": d("c2w", (128, 9, COUT), BF16, kind="ExternalInput").ap(),
        "g1w": d("g1w", (E, COUT), F32, kind="ExternalInput").ap(),
        "g1b": d("g1b", (1, COUT), F32, kind="ExternalInput").ap(),
        "g2w": d("g2w", (E, COUT), F32, kind="ExternalInput").ap(),
        "g2b": d("g2b", (1, COUT), F32, kind="ExternalInput").ap(),
        "dsb": d("dsb", (COUT, 1), F32, kind="ExternalInput").ap(),
    }
    outs = {
        "out": d("out", (COUT, HO, WO), F32, kind="ExternalOutput").ap(),
        "g1o": d("g1o", (1, COUT), F32, kind="ExternalOutput").ap(),
        "g2o": d("g2o", (1, COUT), F32, kind="ExternalOutput").ap(),
    }
    from contextlib import ExitStack
    with tile.TileContext(nc) as tc:
        with ExitStack() as ctx:
            _emit(tc, nc, ins, outs, ctx)
    nc.compile()
    return nc


def prep_weights(conv1_w, conv2_w, ds_w, ds_gamma, ds_beta,
                 gate1_w, gate1_b, gate2_w, gate2_b):
    """Host-side static weight layout prep (same for all cores)."""
    # transposed [co, dx, k] layouts: the kernel gathers the g1-selected
    # out-channel columns on device via one-hot matmuls
    c1aT = np.concatenate(
        [conv1_w[:, :, 1, :], conv1_w[:, :, 0, :]],
        axis=1).transpose(0, 2, 1).astype(NP_BF16)      # [co, dx, 128k]
    w2 = conv1_w[:, :, 2, :]
    c1bT = np.concatenate(
        [np.zeros_like(w2), w2], axis=1).transpose(0, 2, 1).astype(NP_BF16)
    iota96 = np.broadcast_to(np.arange(96, dtype=np.float32) % 32,
                             (128, 96)).copy()
    tri = np.tril(np.ones((128, 128), np.float32)).T.astype(NP_BF16)
    scale = (ds_gamma / np.sqrt(1.0 + EPS)).astype(np.float32)
    dsw = (ds_w[:, :, 0, 0] * scale[:, None]).transpose(1, 0).astype(NP_BF16)
    c2w = np.ascontiguousarray(
        conv2_w.transpose(1, 2, 3, 0).reshape(CIN * 2, 9, COUT)).astype(NP_BF16)
    return {
        "c1aT": np.ascontiguousarray(c1aT),
        "c1bT": np.ascontiguousarray(c1bT),
        "iota96": iota96,
        "tri": np.ascontiguousarray(tri),
        "dsw": np.ascontiguousarray(dsw),
        "c2w": c2w,
        "g1w": np.ascontiguousarray(gate1_w.astype(np.float32)),
        "g1b": np.ascontiguousarray(gate1_b.astype(np.float32)[None, :]),
        "g2w": np.ascontiguousarray(gate2_w.astype(np.float32)),
        "g2b": np.ascontiguousarray(gate2_b.astype(np.float32)[None, :]),
        "dsb": np.ascontiguousarray(ds_beta.astype(np.float32)[:, None]),
    }


def make_in_maps(x, embedding, weights):
    return [
        {"x": np.ascontiguousarray(x[i].astype(np.float32)),
         "emb": np.ascontiguousarray(embedding[i].astype(np.float32)[:, None]),
         **weights}
        for i in range(B)
    ]


_NC_CACHE = {}


def kernel(x, embedding, conv1_w, conv2_w, ds_w, ds_gamma, ds_beta,
           gate1_w, gate1_b, gate2_w, gate2_b, _trace=False):
    x = np.asarray(x)
    embedding = np.asarray(embedding)
    weights = prep_weights(
        np.asarray(conv1_w), np.asarray(conv2_w), np.asarray(ds_w),
        np.asarray(ds_gamma), np.asarray(ds_beta), np.asarray(gate1_w),
        np.asarray(gate1_b), np.asarray(gate2_w), np.asarray(gate2_b))
    if "nc" not in _NC_CACHE:
        _NC_CACHE["nc"] = build_nc()
    nc = _NC_CACHE["nc"]
    in_maps = make_in_maps(x, embedding, weights)
    res = run_bass_kernel_spmd(nc, in_maps, core_ids=list(range(B)),
                               trace=_trace)
    out6 = np.stack([res.results[i]["out"] for i in range(B)])
    g1 = np.stack([res.results[i]["g1o"][0] for i in range(B)])
    g2 = np.stack([res.results[i]["g2o"][0] for i in range(B)])
    kernel.last_results = res
    return (out6.astype(np.float32), g1.astype(np.float32),
            g2.astype(np.float32))
